# revision 19
# baseline (speedup 1.0000x reference)
"""RGCN (mean-aggr) Trainium2 kernel, 8-core SPMD, dst-sharded, bf16.

Strategy (per core, owning a 12544-wide dst range):
  Phase A: 16 dma_gather calls (4 src-windows x 4 dst-subranges, round-robin
    over 4 SWDGE queues) pull the deduplicated edge source rows from x (bf16)
    into SBUF, then write them contiguously to an internal HBM buffer B_s,
    giving each subrange a <=32k-row index window for phase B.
  Phase B: per 448-dst sweep, one dma_gather re-reads that sweep's edge rows
    from B_s in slot-tile-major order. One-hot segment matrices (1/cnt
    weights folded in) are precomputed on the host and DMA-streamed per
    sweep. Per-chunk matmuls (lhsT = gathered rows [128e x 128f] bf16,
    rhs = one-hot slice) accumulate mean^T [feat x slot] into 7 per-bank
    PSUM block tiles; each block is copied to SBUF bf16 by the Scalar engine
    as soon as its 4 tiles finish.
  Slots within a sweep are laid out rel-major (slot = rel*448 + dst_local),
    so the per-relation transform matmuls read contiguous 448-column slices
    of mean^T at full bf16 speed.
  Transform: per sweep, 8 per-relation matmuls (lhsT = W[r] bf16) plus the
    root matmul (lhsT = W_root, rhs = x^T) accumulate out^T [feat x dst] in
    PSUM; bias added on drain (DVE, fp32).
Phase A emission is interleaved with phase B per dst-subrange.
Output is out^T per core; the host transposes and concatenates.
"""

import numpy as np
import ml_dtypes

BF16 = ml_dtypes.bfloat16

P = 128
N_NODES = 100000
N_EDGES = 600000
DIM = 128
NUM_RELS = 8
NCORES = 8

CW = 12544            # dst width per core (8*CW >= N_NODES)
NSUB = 4              # dst subranges per core
SUBW = CW // NSUB     # 3136 dst per subrange
NQ = 4                # src windows
QW = 25088            # src window width (4*QW >= N_NODES, QW < 32768)
TILE_SLOTS = 128      # slot-tile width
TILES_PER_SUB = SUBW * NUM_RELS // TILE_SLOTS  # 196
SWEEP_TILES = 28      # tiles per psum sweep (28*128 = 3584 slots = 448 dst)
SWEEPS_PER_SUB = TILES_PER_SUB // SWEEP_TILES  # 7
SWEEP_DST = SWEEP_TILES * TILE_SLOTS // NUM_RELS  # 448
NQUEUES = 4           # SWDGE queues (Q7 cpu pairs)

_compiled = None


def _wrap16(idx_i16):
    """1-D int16 idx array (len % 16 == 0) -> [128, n/16] wrapped+replicated."""
    n = len(idx_i16)
    w = idx_i16.reshape(n // 16, 16).T  # [16, n/16]
    return np.ascontiguousarray(np.tile(w, (8, 1)))


def _build_program(CAPA, capt, sweep_tok, sweep_chunks):
    import concourse.bacc as bacc
    import concourse.tile as tile
    from concourse import mybir

    TOTB = int(sum(sweep_tok))
    NCHUNKS = int(sum(sweep_chunks))
    BROWS = NQ * CAPA * 2  # paired-node blocks, no zero row
    BLKS = 7                      # psA block tiles per sweep (4 tiles each)
    BLKW = 4 * TILE_SLOTS         # 512 fp32 = one PSUM bank

    nc = bacc.Bacc(None, target_bir_lowering=False, debug=False,
                   num_swdge_queues=NQUEUES,
                   dynamic_dma_scratch_size=65536)
    f32 = mybir.dt.float32
    bf16 = mybir.dt.bfloat16
    i16 = mybir.dt.int16

    xg_d = nc.dram_tensor("xg", [NQ * QW // 2, 2 * P], bf16, kind="ExternalInput")
    xT_d = nc.dram_tensor("xT", [P, CW], bf16, kind="ExternalInput")
    wcat_d = nc.dram_tensor("wcat", [P, NUM_RELS * P], bf16, kind="ExternalInput")
    wroot_d = nc.dram_tensor("wroot", [P, P], bf16, kind="ExternalInput")
    bias_d = nc.dram_tensor("bias", [P, 1], f32, kind="ExternalInput")
    gA_d = nc.dram_tensor("gA", [NSUB * NQ, P, CAPA // 16], i16, kind="ExternalInput")
    gB_d = nc.dram_tensor("gB", [P, TOTB // 16], i16, kind="ExternalInput")
    scm_d = nc.dram_tensor("scm", [P, NCHUNKS, P], bf16, kind="ExternalInput")
    outT_d = nc.dram_tensor("outT", [P, CW], f32, kind="ExternalOutput")

    B_d = [nc.dram_tensor(f"B{s}", [BROWS, P], bf16) for s in range(NSUB)]

    with tile.TileContext(nc) as tc:
        with (
            tc.tile_pool(name="const", bufs=1) as cpool,
            tc.tile_pool(name="stagA", bufs=4) as poolA,
            tc.tile_pool(name="stagB", bufs=4) as poolB,
            tc.tile_pool(name="scp", bufs=3) as scpool,
            tc.tile_pool(name="mpool", bufs=2) as mpool,
            tc.tile_pool(name="opool", bufs=2) as opool,
            tc.tile_pool(name="ipool", bufs=6) as ipool,
            tc.tile_pool(name="psA", bufs=1, space="PSUM") as psA,
            tc.tile_pool(name="psO", bufs=1, space="PSUM") as psO,
        ):
            wcat = cpool.tile([P, NUM_RELS * P], bf16)
            wroot = cpool.tile([P, P], bf16)
            bias = cpool.tile([P, 1], f32)

            nc.sync.dma_start(out=wcat[:], in_=wcat_d[:])
            nc.sync.dma_start(out=wroot[:], in_=wroot_d[:])
            nc.sync.dma_start(out=bias[:], in_=bias_d[:])

            gq = 0  # round-robin SWDGE queue counter

            def phase_a(s):
                """Emit subrange s's src-window gathers -> B_s."""
                nonlocal gq
                for q in range(NQ):
                    gA = ipool.tile([P, CAPA // 16], i16, tag="gA",
                                    name=f"gA{s}_{q}")
                    nc.sync.dma_start(out=gA[:], in_=gA_d[s * NQ + q])
                    stag = poolA.tile([P, CAPA // P, 2 * P], bf16, tag="stagA",
                                      name=f"stA{s}_{q}")
                    nc.gpsimd.dma_gather(
                        out_ap=stag[:],
                        in_ap=xg_d[(QW // 2) * q:(QW // 2) * (q + 1), :],
                        idxs_ap=gA[:],
                        num_idxs=CAPA, num_idxs_reg=CAPA, elem_size=2 * P,
                        single_packet=False, queue_num=gq % NQUEUES)
                    gq += 1
                    nc.sync.dma_start(
                        out=B_d[s][2 * CAPA * q:2 * CAPA * (q + 1), :].rearrange(
                            "(a p two) d -> p a (two d)", p=P, two=2),
                        in_=stag[:])

            sw = 0
            tok_off = 0
            chunk_off = 0
            phase_a(0)
            for s in range(NSUB):
                # software pipeline: next subrange's gathers go in front of
                # this subrange's sweeps so the Q7 pairs stay fed while B_s
                # writes drain
                if s + 1 < NSUB:
                    phase_a(s + 1)

                # ---- Phase B sweeps for subrange s ----
                for k in range(SWEEPS_PER_SUB):
                    swtok = int(sweep_tok[sw])
                    swch = int(sweep_chunks[sw])
                    gB = ipool.tile([P, swtok // 16], i16, tag="gB")
                    nc.sync.dma_start(
                        out=gB[:], in_=gB_d[:, tok_off // 16:(tok_off + swtok) // 16])
                    stag = poolB.tile([P, swtok // P, P], bf16, tag="stagB")
                    nc.gpsimd.dma_gather(
                        out_ap=stag[:], in_ap=B_d[s][:, :], idxs_ap=gB[:],
                        num_idxs=swtok, num_idxs_reg=swtok, elem_size=P,
                        single_packet=False, queue_num=gq % NQUEUES)
                    gq += 1

                    scm = scpool.tile([P, swch, P], bf16, tag="scm")
                    nc.scalar.dma_start(
                        out=scm[:], in_=scm_d[:, chunk_off:chunk_off + swch, :])

                    meanT = mpool.tile([P, SWEEP_TILES * TILE_SLOTS], bf16,
                                       tag="meanT")
                    blk_tiles = [
                        psA.tile([P, BLKW], f32, name=f"blk{b}", tag=f"blk{b}")
                        for b in range(BLKS)]
                    ch = 0
                    for tl in range(SWEEP_TILES):
                        t_glob = s * TILES_PER_SUB + k * SWEEP_TILES + tl
                        nch = int(capt[t_glob]) // P
                        b, off = tl // 4, (tl % 4) * TILE_SLOTS
                        for j in range(nch):
                            nc.tensor.matmul(
                                out=blk_tiles[b][:, off:off + TILE_SLOTS],
                                lhsT=stag[:, ch, :], rhs=scm[:, ch, :],
                                start=(j == 0), stop=(j == nch - 1))
                            ch += 1
                        if tl % 4 == 3:  # block complete -> drain to SBUF
                            nc.scalar.copy(
                                out=meanT[:, b * BLKW:(b + 1) * BLKW],
                                in_=blk_tiles[b][:])
                    assert ch == swch

                    dst0 = s * SUBW + k * SWEEP_DST
                    xTt = ipool.tile([P, SWEEP_DST], bf16, tag="xT")
                    nc.sync.dma_start(out=xTt[:], in_=xT_d[:, dst0:dst0 + SWEEP_DST])
                    outp = psO.tile([P, SWEEP_DST], f32)
                    for r in range(NUM_RELS):
                        nc.tensor.matmul(
                            out=outp[:], lhsT=wcat[:, r * P:(r + 1) * P],
                            rhs=meanT[:, r * SWEEP_DST:(r + 1) * SWEEP_DST],
                            start=(r == 0), stop=False)
                    nc.tensor.matmul(out=outp[:], lhsT=wroot[:], rhs=xTt[:],
                                     start=False, stop=True)
                    oT = opool.tile([P, SWEEP_DST], f32, tag="oT")
                    nc.vector.tensor_scalar_add(out=oT[:], in0=outp[:], scalar1=bias[:])
                    nc.sync.dma_start(out=outT_d[:, dst0:dst0 + SWEEP_DST], in_=oT[:])

                    tok_off += swtok
                    chunk_off += swch
                    sw += 1
    nc.compile()
    return nc


def _prepare(x, W, W_root, bias, edge_index, edge_type):
    src = np.asarray(edge_index[0], dtype=np.int64)
    dst = np.asarray(edge_index[1], dtype=np.int64)
    rel = np.asarray(edge_type, dtype=np.int64)

    cnt = np.bincount(dst * NUM_RELS + rel, minlength=N_NODES * NUM_RELS)
    w_edge = (1.0 / np.maximum(cnt[dst * NUM_RELS + rel], 1)).astype(np.float32)

    core = dst // CW
    dst_local = dst - core * CW
    # rel-major slot layout within each sweep: transform reads contiguous
    sweep_g = dst_local // SWEEP_DST
    dloc = dst_local % SWEEP_DST
    slot_sw = rel * SWEEP_DST + dloc
    tile_g = sweep_g * SWEEP_TILES + slot_sw // TILE_SLOTS
    scol_val = slot_sw % TILE_SLOTS
    sub = dst_local // SUBW
    q = src // QW
    gslot = tile_g * TILE_SLOTS + scol_val

    # ---- caps (phase A counts deduplicated 2-node blocks per bucket) ----
    keyA = ((core * NSUB + sub) * NQ + q) * (QW // 2) + (src - q * QW) // 2
    bincA = np.bincount(
        np.unique(keyA) // (QW // 2), minlength=NCORES * NSUB * NQ)
    CAPA = int(-(-bincA.max() // P) * P)
    CAPA = max(CAPA, P)
    assert NQ * CAPA * 2 <= 32768, CAPA  # B rows must stay int16-addressable
    keyT = core * (NSUB * TILES_PER_SUB) + tile_g
    bincT = np.bincount(keyT, minlength=NCORES * NSUB * TILES_PER_SUB).reshape(
        NCORES, NSUB * TILES_PER_SUB)
    capt = (-(-bincT.max(axis=0) // P) * P).astype(np.int64)
    capt = np.maximum(capt, P)

    ntile = NSUB * TILES_PER_SUB
    sweep_tok = capt.reshape(ntile // SWEEP_TILES, SWEEP_TILES).sum(axis=1)
    sweep_chunks = sweep_tok // P
    TOTB = int(sweep_tok.sum())
    NCHUNKS = int(sweep_chunks.sum())
    tile_tok_off = np.concatenate([[0], np.cumsum(capt)])[:-1]

    # ---- shared host arrays ----
    order = np.lexsort((q, gslot, core))  # group by core, then tile/slot, then q
    in_maps = []
    xg = np.zeros((NQ * QW, P), BF16)
    xg[:N_NODES] = np.asarray(x, np.float32).astype(BF16)
    xg = xg.reshape(NQ * QW // 2, 2 * P)
    wcat = np.ascontiguousarray(
        np.asarray(W, np.float32).transpose(1, 0, 2).reshape(P, NUM_RELS * P)
    ).astype(BF16)
    wroot = np.ascontiguousarray(np.asarray(W_root, np.float32)).astype(BF16)
    biascol = np.asarray(bias, np.float32).reshape(P, 1)

    for c in range(NCORES):
        sel = order[core[order] == c]
        csrc, cq, csub, ctile, cscol, cw = (
            src[sel], q[sel], sub[sel], tile_g[sel], scol_val[sel], w_edge[sel])

        # phase A: bucket by (sub, q); dedup 2-node blocks within each bucket
        keyaq = csub * NQ + cq
        ordA = np.argsort(keyaq, kind="stable")
        gA = np.zeros((NSUB * NQ, P, CAPA // 16), np.int16)
        rankA = np.zeros(len(sel), np.int64)
        for sq in range(NSUB * NQ):
            members = ordA[keyaq[ordA] == sq]
            vals = (csrc[members] - QW * (sq % NQ)).astype(np.int32)
            blocks = np.unique(vals // 2)
            n = len(blocks)
            assert n <= CAPA, (n, CAPA)
            rankA[members] = 2 * np.searchsorted(blocks, vals // 2) + (vals & 1)
            idx = np.zeros(CAPA, np.int16)
            idx[:n] = blocks.astype(np.int16)
            gA[sq] = _wrap16(idx)
        brow = 2 * CAPA * cq + rankA  # B_s row for each edge

        # phase B: token layout, tile-major with per-tile caps
        # pads point at row 0 (real finite data; their scm columns are zero)
        gB_lin = np.zeros(TOTB, np.int16)
        ordT = np.argsort(ctile, kind="stable")
        tcounts = np.bincount(ctile, minlength=ntile)
        tstart = np.concatenate([[0], np.cumsum(tcounts)])[:-1]
        arangepos = np.empty(len(sel), np.int64)
        arangepos[ordT] = np.arange(len(sel))
        rank_in_tile = arangepos - tstart[ctile]
        tok = tile_tok_off[ctile] + rank_in_tile
        assert (rank_in_tile < capt[ctile]).all()
        gB_lin[tok] = brow.astype(np.int16)

        # host-built one-hot segment matrices (weights folded in)
        scm = np.zeros((NCHUNKS * P, P), np.float32)
        scm[tok, cscol] = cw
        scm = np.ascontiguousarray(
            scm.reshape(NCHUNKS, P, P).transpose(1, 0, 2)).astype(BF16)

        xT = np.zeros((P, CW), BF16)
        lo, hi = CW * c, min(CW * (c + 1), N_NODES)
        xT[:, :hi - lo] = np.asarray(x[lo:hi], np.float32).astype(BF16).T

        in_maps.append({
            "xg": xg,
            "xT": xT,
            "wcat": wcat,
            "wroot": wroot,
            "bias": biascol,
            "gA": gA,
            "gB": _wrap16(gB_lin),
            "scm": scm,
        })
    return in_maps, CAPA, capt, sweep_tok, sweep_chunks


LAST_EXEC_NS = None


def kernel(x, W, W_root, bias, edge_index, edge_type):
    global _compiled, LAST_EXEC_NS
    import os
    from concourse.bass_utils import run_bass_kernel_spmd

    in_maps, CAPA, capt, sweep_tok, sweep_chunks = _prepare(
        x, W, W_root, bias, edge_index, edge_type)
    key = (CAPA, capt.tobytes())
    if _compiled is None or _compiled[0] != key:
        nc = _build_program(CAPA, capt, sweep_tok, sweep_chunks)
        _compiled = (key, nc)
    nc = _compiled[1]

    trace = bool(int(os.environ.get("BASS_PROFILE", "0")))
    r = run_bass_kernel_spmd(nc, in_maps, list(range(NCORES)), trace=trace)
    if trace:
        LAST_EXEC_NS = r.exec_time_ns
    res = r.results
    out = np.empty((NCORES * CW, DIM), np.float32)
    for c in range(NCORES):
        out[CW * c:CW * (c + 1)] = res[c]["outT"].T
    return out[:N_NODES]


# revision 21
# speedup vs baseline: 1.2714x; 1.2714x over previous
"""RGCN (mean-aggr) Trainium2 kernel, 8-core SPMD, dst-sharded, bf16.

Strategy (per core, owning a 12544-wide dst range):
  Phase A: 16 dma_gather calls (4 src-windows x 4 dst-subranges, round-robin
    over 4 SWDGE queues) pull the deduplicated edge source rows from x (bf16)
    into SBUF, then write them contiguously to an internal HBM buffer B_s,
    giving each subrange a <=32k-row index window for phase B.
  Phase B: per 448-dst sweep, one dma_gather re-reads that sweep's edge rows
    from B_s in slot-tile-major order. One-hot segment matrices (1/cnt
    weights folded in) are precomputed on the host and DMA-streamed per
    sweep. Per-chunk matmuls (lhsT = gathered rows [128e x 128f] bf16,
    rhs = one-hot slice) accumulate mean^T [feat x slot] into 7 per-bank
    PSUM block tiles; each block is copied to SBUF bf16 by the Scalar engine
    as soon as its 4 tiles finish.
  Slots within a sweep are laid out rel-major (slot = rel*448 + dst_local),
    so the per-relation transform matmuls read contiguous 448-column slices
    of mean^T at full bf16 speed.
  Transform: per sweep, 8 per-relation matmuls (lhsT = W[r] bf16) plus the
    root matmul (lhsT = W_root, rhs = x^T) accumulate out^T [feat x dst] in
    PSUM; bias added on drain (DVE, fp32).
Phase A emission is interleaved with phase B per dst-subrange.
Output is out^T per core; the host transposes and concatenates.
"""

import numpy as np
import ml_dtypes

BF16 = ml_dtypes.bfloat16

P = 128
N_NODES = 100000
N_EDGES = 600000
DIM = 128
NUM_RELS = 8
NCORES = 8

CW = 12544            # dst width per core (8*CW >= N_NODES)
NSUB = 4              # dst subranges per core
SUBW = CW // NSUB     # 3136 dst per subrange
NQ = 4                # src windows
QW = 25088            # src window width (4*QW >= N_NODES, QW < 32768)
TILE_SLOTS = 128      # slot-tile width
TILES_PER_SUB = SUBW * NUM_RELS // TILE_SLOTS  # 196
SWEEP_TILES = 28      # tiles per psum sweep (28*128 = 3584 slots = 448 dst)
SWEEPS_PER_SUB = TILES_PER_SUB // SWEEP_TILES  # 7
SWEEP_DST = SWEEP_TILES * TILE_SLOTS // NUM_RELS  # 448
NQUEUES = 4           # SWDGE queues (Q7 cpu pairs)

_compiled = None


def _wrap16(idx_i16):
    """1-D int16 idx array (len % 16 == 0) -> [128, n/16] wrapped+replicated."""
    n = len(idx_i16)
    w = idx_i16.reshape(n // 16, 16).T  # [16, n/16]
    return np.ascontiguousarray(np.tile(w, (8, 1)))


def _build_program(CAPA, capt, sweep_tok, sweep_chunks):
    import concourse.bacc as bacc
    import concourse.tile as tile
    from concourse import mybir

    TOTB = int(sum(sweep_tok))
    NCHUNKS = int(sum(sweep_chunks))
    BROWS = NQ * CAPA + P
    BLKS = 7                      # psA block tiles per sweep (4 tiles each)
    BLKW = 4 * TILE_SLOTS         # 512 fp32 = one PSUM bank

    nc = bacc.Bacc(None, target_bir_lowering=False, debug=False,
                   num_swdge_queues=NQUEUES,
                   dynamic_dma_scratch_size=65536)
    f32 = mybir.dt.float32
    bf16 = mybir.dt.bfloat16
    i16 = mybir.dt.int16

    xg_d = nc.dram_tensor("xg", [NQ * QW, P], bf16, kind="ExternalInput")
    xT_d = nc.dram_tensor("xT", [P, CW], bf16, kind="ExternalInput")
    wcat_d = nc.dram_tensor("wcat", [P, NUM_RELS * P], bf16, kind="ExternalInput")
    wroot_d = nc.dram_tensor("wroot", [P, P], bf16, kind="ExternalInput")
    bias_d = nc.dram_tensor("bias", [P, 1], f32, kind="ExternalInput")
    gA_d = nc.dram_tensor("gA", [NSUB * NQ, P, CAPA // 16], i16, kind="ExternalInput")
    gB_d = nc.dram_tensor("gB", [P, TOTB // 16], i16, kind="ExternalInput")
    scm_d = nc.dram_tensor("scm", [P, NCHUNKS, P], bf16, kind="ExternalInput")
    outT_d = nc.dram_tensor("outT", [P, CW], f32, kind="ExternalOutput")

    B_d = [nc.dram_tensor(f"B{s}", [BROWS, P], bf16) for s in range(NSUB)]

    with tile.TileContext(nc) as tc:
        with (
            tc.tile_pool(name="const", bufs=1) as cpool,
            tc.tile_pool(name="stagA", bufs=5) as poolA,
            tc.tile_pool(name="stagB", bufs=4) as poolB,
            tc.tile_pool(name="scp", bufs=3) as scpool,
            tc.tile_pool(name="mpool", bufs=2) as mpool,
            tc.tile_pool(name="opool", bufs=2) as opool,
            tc.tile_pool(name="ipool", bufs=12) as ipool,
            tc.tile_pool(name="psA", bufs=1, space="PSUM") as psA,
            tc.tile_pool(name="psO", bufs=1, space="PSUM") as psO,
        ):
            wcat = cpool.tile([P, NUM_RELS * P], bf16)
            wroot = cpool.tile([P, P], bf16)
            bias = cpool.tile([P, 1], f32)
            zrow = cpool.tile([P, P], bf16)

            nc.sync.dma_start(out=wcat[:], in_=wcat_d[:])
            nc.sync.dma_start(out=wroot[:], in_=wroot_d[:])
            nc.sync.dma_start(out=bias[:], in_=bias_d[:])
            nc.vector.memset(zrow[:], 0.0)

            gq = 0  # round-robin SWDGE queue counter

            def phase_a(s):
                """Emit subrange s's src-window gathers -> B_s."""
                nonlocal gq
                nc.sync.dma_start(
                    out=B_d[s][NQ * CAPA:NQ * CAPA + P, :], in_=zrow[:])
                for q in range(NQ):
                    gA = ipool.tile([P, CAPA // 16], i16, tag="gA",
                                    name=f"gA{s}_{q}")
                    nc.sync.dma_start(out=gA[:], in_=gA_d[s * NQ + q])
                    stag = poolA.tile([P, CAPA // P, P], bf16, tag="stagA",
                                      name=f"stA{s}_{q}")
                    nc.gpsimd.dma_gather(
                        out_ap=stag[:],
                        in_ap=xg_d[QW * q:QW * (q + 1), :],
                        idxs_ap=gA[:],
                        num_idxs=CAPA, num_idxs_reg=CAPA, elem_size=P,
                        single_packet=False, queue_num=gq % NQUEUES)
                    gq += 1
                    nc.sync.dma_start(
                        out=B_d[s][CAPA * q:CAPA * (q + 1), :].rearrange(
                            "(a p) d -> p a d", p=P),
                        in_=stag[:])

            sw = 0
            tok_off = 0
            chunk_off = 0
            phase_a(0)
            for s in range(NSUB):
                # software pipeline: next subrange's gathers go in front of
                # this subrange's sweeps so the Q7 pairs stay fed while B_s
                # writes drain
                if s + 1 < NSUB:
                    phase_a(s + 1)

                # ---- Phase B sweeps for subrange s ----
                for k in range(SWEEPS_PER_SUB):
                    swtok = int(sweep_tok[sw])
                    swch = int(sweep_chunks[sw])
                    gB = ipool.tile([P, swtok // 16], i16, tag="gB")
                    nc.sync.dma_start(
                        out=gB[:], in_=gB_d[:, tok_off // 16:(tok_off + swtok) // 16])
                    stag = poolB.tile([P, swtok // P, P], bf16, tag="stagB")
                    nc.gpsimd.dma_gather(
                        out_ap=stag[:], in_ap=B_d[s][:, :], idxs_ap=gB[:],
                        num_idxs=swtok, num_idxs_reg=swtok, elem_size=P,
                        single_packet=False, queue_num=gq % NQUEUES)
                    gq += 1

                    scm = scpool.tile([P, swch, P], bf16, tag="scm")
                    nc.scalar.dma_start(
                        out=scm[:], in_=scm_d[:, chunk_off:chunk_off + swch, :])

                    meanT = mpool.tile([P, SWEEP_TILES * TILE_SLOTS], bf16,
                                       tag="meanT")
                    blk_tiles = [
                        psA.tile([P, BLKW], f32, name=f"blk{b}", tag=f"blk{b}")
                        for b in range(BLKS)]
                    ch = 0
                    for tl in range(SWEEP_TILES):
                        t_glob = s * TILES_PER_SUB + k * SWEEP_TILES + tl
                        nch = int(capt[t_glob]) // P
                        b, off = tl // 4, (tl % 4) * TILE_SLOTS
                        for j in range(nch):
                            nc.tensor.matmul(
                                out=blk_tiles[b][:, off:off + TILE_SLOTS],
                                lhsT=stag[:, ch, :], rhs=scm[:, ch, :],
                                start=(j == 0), stop=(j == nch - 1))
                            ch += 1
                        if tl % 4 == 3:  # block complete -> drain to SBUF
                            nc.scalar.copy(
                                out=meanT[:, b * BLKW:(b + 1) * BLKW],
                                in_=blk_tiles[b][:])
                    assert ch == swch

                    dst0 = s * SUBW + k * SWEEP_DST
                    xTt = ipool.tile([P, SWEEP_DST], bf16, tag="xT")
                    nc.sync.dma_start(out=xTt[:], in_=xT_d[:, dst0:dst0 + SWEEP_DST])
                    outp = psO.tile([P, SWEEP_DST], f32)
                    for r in range(NUM_RELS):
                        nc.tensor.matmul(
                            out=outp[:], lhsT=wcat[:, r * P:(r + 1) * P],
                            rhs=meanT[:, r * SWEEP_DST:(r + 1) * SWEEP_DST],
                            start=(r == 0), stop=False)
                    nc.tensor.matmul(out=outp[:], lhsT=wroot[:], rhs=xTt[:],
                                     start=False, stop=True)
                    oT = opool.tile([P, SWEEP_DST], f32, tag="oT")
                    nc.vector.tensor_scalar_add(out=oT[:], in0=outp[:], scalar1=bias[:])
                    nc.sync.dma_start(out=outT_d[:, dst0:dst0 + SWEEP_DST], in_=oT[:])

                    tok_off += swtok
                    chunk_off += swch
                    sw += 1
    nc.compile()
    return nc


def _prepare(x, W, W_root, bias, edge_index, edge_type):
    src = np.asarray(edge_index[0], dtype=np.int64)
    dst = np.asarray(edge_index[1], dtype=np.int64)
    rel = np.asarray(edge_type, dtype=np.int64)

    cnt = np.bincount(dst * NUM_RELS + rel, minlength=N_NODES * NUM_RELS)
    w_edge = (1.0 / np.maximum(cnt[dst * NUM_RELS + rel], 1)).astype(np.float32)

    core = dst // CW
    dst_local = dst - core * CW
    # rel-major slot layout within each sweep: transform reads contiguous
    sweep_g = dst_local // SWEEP_DST
    dloc = dst_local % SWEEP_DST
    slot_sw = rel * SWEEP_DST + dloc
    tile_g = sweep_g * SWEEP_TILES + slot_sw // TILE_SLOTS
    scol_val = slot_sw % TILE_SLOTS
    sub = dst_local // SUBW
    q = src // QW
    gslot = tile_g * TILE_SLOTS + scol_val

    # ---- caps (phase A counts deduplicated per bucket) ----
    keyA = ((core * NSUB + sub) * NQ + q) * QW + (src - q * QW)
    bincA = np.bincount(
        np.unique(keyA) // QW, minlength=NCORES * NSUB * NQ)
    CAPA = int(-(-bincA.max() // P) * P)
    CAPA = max(CAPA, P)
    keyT = core * (NSUB * TILES_PER_SUB) + tile_g
    bincT = np.bincount(keyT, minlength=NCORES * NSUB * TILES_PER_SUB).reshape(
        NCORES, NSUB * TILES_PER_SUB)
    capt = (-(-bincT.max(axis=0) // P) * P).astype(np.int64)
    capt = np.maximum(capt, P)

    ntile = NSUB * TILES_PER_SUB
    sweep_tok = capt.reshape(ntile // SWEEP_TILES, SWEEP_TILES).sum(axis=1)
    sweep_chunks = sweep_tok // P
    TOTB = int(sweep_tok.sum())
    NCHUNKS = int(sweep_chunks.sum())
    tile_tok_off = np.concatenate([[0], np.cumsum(capt)])[:-1]

    # ---- shared host arrays ----
    order = np.lexsort((q, gslot, core))  # group by core, then tile/slot, then q
    in_maps = []
    xg = np.zeros((NQ * QW, P), BF16)
    xg[:N_NODES] = np.asarray(x, np.float32).astype(BF16)
    wcat = np.ascontiguousarray(
        np.asarray(W, np.float32).transpose(1, 0, 2).reshape(P, NUM_RELS * P)
    ).astype(BF16)
    wroot = np.ascontiguousarray(np.asarray(W_root, np.float32)).astype(BF16)
    biascol = np.asarray(bias, np.float32).reshape(P, 1)

    for c in range(NCORES):
        sel = order[core[order] == c]
        csrc, cq, csub, ctile, cscol, cw = (
            src[sel], q[sel], sub[sel], tile_g[sel], scol_val[sel], w_edge[sel])

        # phase A: bucket by (sub, q); dedup nodes within each bucket
        keyaq = csub * NQ + cq
        ordA = np.argsort(keyaq, kind="stable")
        gA = np.zeros((NSUB * NQ, P, CAPA // 16), np.int16)
        rankA = np.zeros(len(sel), np.int64)
        for sq in range(NSUB * NQ):
            members = ordA[keyaq[ordA] == sq]
            vals = (csrc[members] - QW * (sq % NQ)).astype(np.int16)
            uniq, inv = np.unique(vals, return_inverse=True)
            n = len(uniq)
            assert n <= CAPA, (n, CAPA)
            rankA[members] = inv
            idx = np.zeros(CAPA, np.int16)
            idx[:n] = uniq
            gA[sq] = _wrap16(idx)
        brow = CAPA * cq + rankA  # B_s row for each edge

        # phase B: token layout, tile-major with per-tile caps
        gB_lin = np.zeros(TOTB, np.int16)
        zr = (NQ * CAPA + (np.arange(TOTB) % P)).astype(np.int16)
        gB_lin[:] = zr
        ordT = np.argsort(ctile, kind="stable")
        tcounts = np.bincount(ctile, minlength=ntile)
        tstart = np.concatenate([[0], np.cumsum(tcounts)])[:-1]
        arangepos = np.empty(len(sel), np.int64)
        arangepos[ordT] = np.arange(len(sel))
        rank_in_tile = arangepos - tstart[ctile]
        tok = tile_tok_off[ctile] + rank_in_tile
        assert (rank_in_tile < capt[ctile]).all()
        gB_lin[tok] = brow.astype(np.int16)

        # host-built one-hot segment matrices (weights folded in)
        scm = np.zeros((NCHUNKS * P, P), np.float32)
        scm[tok, cscol] = cw
        scm = np.ascontiguousarray(
            scm.reshape(NCHUNKS, P, P).transpose(1, 0, 2)).astype(BF16)

        xT = np.zeros((P, CW), BF16)
        lo, hi = CW * c, min(CW * (c + 1), N_NODES)
        xT[:, :hi - lo] = np.asarray(x[lo:hi], np.float32).astype(BF16).T

        in_maps.append({
            "xg": xg,
            "xT": xT,
            "wcat": wcat,
            "wroot": wroot,
            "bias": biascol,
            "gA": gA,
            "gB": _wrap16(gB_lin),
            "scm": scm,
        })
    return in_maps, CAPA, capt, sweep_tok, sweep_chunks


LAST_EXEC_NS = None


def kernel(x, W, W_root, bias, edge_index, edge_type):
    global _compiled, LAST_EXEC_NS
    import os
    from concourse.bass_utils import run_bass_kernel_spmd

    in_maps, CAPA, capt, sweep_tok, sweep_chunks = _prepare(
        x, W, W_root, bias, edge_index, edge_type)
    key = (CAPA, capt.tobytes())
    if _compiled is None or _compiled[0] != key:
        nc = _build_program(CAPA, capt, sweep_tok, sweep_chunks)
        _compiled = (key, nc)
    nc = _compiled[1]

    trace = bool(int(os.environ.get("BASS_PROFILE", "0")))
    r = run_bass_kernel_spmd(nc, in_maps, list(range(NCORES)), trace=trace)
    if trace:
        LAST_EXEC_NS = r.exec_time_ns
    res = r.results
    out = np.empty((NCORES * CW, DIM), np.float32)
    for c in range(NCORES):
        out[CW * c:CW * (c + 1)] = res[c]["outT"].T
    return out[:N_NODES]
